# revision 6
# baseline (speedup 1.0000x reference)
"""MoE DynamicGate kernel for 8x Trainium2 NeuronCores.

Data-parallel over the token dim (16384 tokens -> 8 shards of 2048); all gate
parameters are tiny and replicated to every core.

Math (eval-mode forward, matching the nn.Module reference):
    h   = x @ Wp + bp                      [N, 256]   (viewed as 4 heads x 64)
    hid = relu(h @ blockdiag(W1) + b1)     [N, 256]
    cmb = hid @ blockdiag(W2) + b2         [N, 256]
    gl  = (cmb @ Wc + bc) / clip(T,.5,5)   [N, 64]
    top2 + softmax-over-2 + scatter

Since relu is the only nonlinearity, the host folds the linear pieces:
    Wpf = Wp @ blockdiag(W1)       bpf = bp @ blockdiag(W1) + b1
    Wf  = blockdiag(W2) @ Wc / T   bcf = (b2 @ Wc + bc) / T
so the device computes:  gl = relu(x @ Wpf + bpf) @ Wf + bcf
(T = 2.0 is a power of two so folding 1/T into Wf/bcf is bit-exact; for other
T it only perturbs at the 2^-24 level.)

The device works feature-major (x^T is prepared host-side) so every matmul has
its contraction dim on SBUF partitions, biases are per-partition (fused into
the ACT relu), and only the tiny [64, 512] logit tiles need a PE transpose
back to token-major for the top-k stage.
"""

import numpy as np

N_TOKENS = 16384
INPUT_DIM = 2048
N_CORES = 8
N_SHARD = N_TOKENS // N_CORES  # 2048
CHUNK = 512                    # tokens per pipeline chunk
N_CHUNKS = N_SHARD // CHUNK    # 4
KT = INPUT_DIM // 128          # 16 contraction tiles
F = 256                        # fused hidden width (4 heads x 64)
E = 64                         # experts
BIG = 1000.0

_CACHE = {}


def _build_bass():
    """Build the per-core Bass/Tile program (SPMD: identical on all cores)."""
    from contextlib import ExitStack

    import concourse.bass as bass
    import concourse.tile as tile
    from concourse import bacc, mybir

    fp32 = mybir.dt.float32
    i32 = mybir.dt.int32
    AF = mybir.ActivationFunctionType
    Alu = mybir.AluOpType
    AX = mybir.AxisListType

    nc = bacc.Bacc(
        "TRN2", target_bir_lowering=False, debug=False, num_devices=N_CORES
    )

    # DRAM I/O (host pre-arranges weight layouts for single clean DMAs)
    xt_d = nc.dram_tensor("xt", [INPUT_DIM, N_SHARD], fp32, kind="ExternalInput")
    wpf_d = nc.dram_tensor("wpf", [128, KT, 2, 128], fp32, kind="ExternalInput")
    b1f_d = nc.dram_tensor("b1f", [128, 2], fp32, kind="ExternalInput")
    wf_d = nc.dram_tensor("wf", [128, 2, E], fp32, kind="ExternalInput")
    bcf_d = nc.dram_tensor("bcf", [E, 1], fp32, kind="ExternalInput")
    iota_d = nc.dram_tensor("iota", [128, N_CHUNKS, E], fp32, kind="ExternalInput")
    id_d = nc.dram_tensor("ident", [E, E], fp32, kind="ExternalInput")

    gates_d = nc.dram_tensor("gates", [N_SHARD, E], fp32, kind="ExternalOutput")
    gl_d = nc.dram_tensor("gl", [N_SHARD, E], fp32, kind="ExternalOutput")
    idx_d = nc.dram_tensor("idx", [N_SHARD, 2], i32, kind="ExternalOutput")

    with tile.TileContext(nc) as tc, ExitStack() as ctx:
        consts = ctx.enter_context(tc.tile_pool(name="consts", bufs=1))
        xtp = ctx.enter_context(tc.tile_pool(name="xt", bufs=2))
        hidp = ctx.enter_context(tc.tile_pool(name="hid", bufs=2))
        glp = ctx.enter_context(tc.tile_pool(name="gl", bufs=2))
        tokp = ctx.enter_context(tc.tile_pool(name="tok", bufs=2))
        outp = ctx.enter_context(tc.tile_pool(name="outs", bufs=2))
        smallp = ctx.enter_context(tc.tile_pool(name="small", bufs=2))
        psA = ctx.enter_context(
            tc.tile_pool(name="psA", bufs=4, space=bass.MemorySpace.PSUM)
        )
        psG = ctx.enter_context(
            tc.tile_pool(name="psG", bufs=2, space=bass.MemorySpace.PSUM)
        )
        psT = ctx.enter_context(
            tc.tile_pool(name="psT", bufs=2, space=bass.MemorySpace.PSUM)
        )

        # constant loads
        wpf_sb = consts.tile([128, KT, 2, 128], fp32)
        nc.sync.dma_start(wpf_sb[:], wpf_d[:])
        b1f_sb = consts.tile([128, 2], fp32)
        nc.sync.dma_start(b1f_sb[:], b1f_d[:])
        wf_sb = consts.tile([128, 2, E], fp32)
        nc.sync.dma_start(wf_sb[:], wf_d[:])
        bcf_sb = consts.tile([E, 1], fp32)
        nc.sync.dma_start(bcf_sb[:], bcf_d[:])
        iota_sb = consts.tile([128, N_CHUNKS, E], fp32)
        nc.sync.dma_start(iota_sb[:], iota_d[:])
        id_sb = consts.tile([E, E], fp32)
        nc.sync.dma_start(id_sb[:], id_d[:])

        for c in range(N_CHUNKS):
            tok_sl = slice(c * CHUNK, (c + 1) * CHUNK)

            # x^T chunk: [128 part (d), 16 ktiles, 512 tokens]
            xt = xtp.tile([128, KT, CHUNK], fp32, tag="xt")
            nc.sync.dma_start(
                xt[:], xt_d[:, tok_sl].rearrange("(k p) t -> p k t", p=128)
            )

            # hid_pre^T = (x @ Wpf)^T by 128-feat halves, K-contiguous
            hids = []
            for m in range(2):
                ps = psA.tile([128, CHUNK], fp32, tag="mm")
                for k in range(KT):
                    nc.tensor.matmul(
                        ps[:],
                        wpf_sb[:, k, m, :],
                        xt[:, k, :],
                        start=(k == 0),
                        stop=(k == KT - 1),
                    )
                hid = hidp.tile([128, CHUNK], fp32, tag=f"hid{m}")
                nc.scalar.activation(
                    hid[:], ps[:], AF.Relu, bias=b1f_sb[:, m : m + 1]
                )
                hids.append(hid)

            # gate logits^T = hid^T' @ Wf + bcf   -> [64, 512]
            pg = psG.tile([E, CHUNK], fp32, tag="pg")
            nc.tensor.matmul(pg[:], wf_sb[:, 0, :], hids[0][:], start=True, stop=False)
            nc.tensor.matmul(pg[:], wf_sb[:, 1, :], hids[1][:], start=False, stop=True)
            gl_sb = glp.tile([E, CHUNK], fp32, tag="gl")
            nc.vector.tensor_scalar_add(gl_sb[:], pg[:], bcf_sb[:])

            # transpose to token-major [128 tokens, 4 j, 64 experts]
            pt = psT.tile([128, N_CHUNKS, E], fp32, tag="pt")
            for j in range(N_CHUNKS):
                nc.tensor.transpose(
                    pt[:, j, :], gl_sb[:, j * 128 : (j + 1) * 128], id_sb[:]
                )
            gl_tok = tokp.tile([128, N_CHUNKS, E], fp32, tag="gltok")
            nc.scalar.copy(gl_tok[:], pt[:])
            nc.sync.dma_start(
                gl_d[tok_sl, :].rearrange("(j p) e -> p j e", p=128), gl_tok[:]
            )

            # ---- top-2 + softmax + scatter-free gates ----
            L = gl_tok[:]
            m1 = smallp.tile([128, N_CHUNKS], fp32, tag="m1")
            nc.vector.reduce_max(m1[:], L, axis=AX.X)
            m1b = m1[:].unsqueeze(2).broadcast_to((128, N_CHUNKS, E))
            mask1 = tokp.tile([128, N_CHUNKS, E], fp32, tag="mask1")
            nc.vector.tensor_tensor(mask1[:], L, m1b, op=Alu.is_ge)

            L2 = tokp.tile([128, N_CHUNKS, E], fp32, tag="L2")
            nc.vector.scalar_tensor_tensor(
                L2[:], mask1[:], -BIG, L, op0=Alu.mult, op1=Alu.add
            )
            m2 = smallp.tile([128, N_CHUNKS], fp32, tag="m2")
            nc.vector.reduce_max(m2[:], L2[:], axis=AX.X)
            m2b = m2[:].unsqueeze(2).broadcast_to((128, N_CHUNKS, E))
            mask2 = tokp.tile([128, N_CHUNKS, E], fp32, tag="mask2")
            nc.vector.tensor_tensor(mask2[:], L2[:], m2b, op=Alu.is_ge)

            # argmax indices: min over (iota + BIG - BIG*mask)
            t1 = tokp.tile([128, N_CHUNKS, E], fp32, tag="t1")
            nc.vector.scalar_tensor_tensor(
                t1[:], mask1[:], -BIG, iota_sb[:], op0=Alu.mult, op1=Alu.add
            )
            i1 = smallp.tile([128, N_CHUNKS], fp32, tag="i1")
            nc.vector.tensor_reduce(i1[:], t1[:], axis=AX.X, op=Alu.min)
            t2 = tokp.tile([128, N_CHUNKS, E], fp32, tag="t2")
            nc.vector.scalar_tensor_tensor(
                t2[:], mask2[:], -BIG, iota_sb[:], op0=Alu.mult, op1=Alu.add
            )
            i2 = smallp.tile([128, N_CHUNKS], fp32, tag="i2")
            nc.vector.tensor_reduce(i2[:], t2[:], axis=AX.X, op=Alu.min)

            # softmax over the two top values: p1 = 1/(1+exp(m2-m1)), p2 = e*p1
            d = smallp.tile([128, N_CHUNKS], fp32, tag="d")
            nc.vector.tensor_tensor(d[:], m2[:], m1[:], op=Alu.subtract)
            ex = smallp.tile([128, N_CHUNKS], fp32, tag="ex")
            nc.scalar.activation(ex[:], d[:], AF.Exp)
            den = smallp.tile([128, N_CHUNKS], fp32, tag="den")
            nc.vector.tensor_scalar_add(den[:], ex[:], 1.0)
            p1 = smallp.tile([128, N_CHUNKS], fp32, tag="p1")
            nc.vector.reciprocal(p1[:], den[:])
            p2 = smallp.tile([128, N_CHUNKS], fp32, tag="p2")
            nc.vector.tensor_tensor(p2[:], ex[:], p1[:], op=Alu.mult)

            g1 = tokp.tile([128, N_CHUNKS, E], fp32, tag="g1")
            nc.vector.tensor_tensor(
                g1[:], mask1[:], p1[:].unsqueeze(2).broadcast_to((128, N_CHUNKS, E)),
                op=Alu.mult,
            )
            g2 = tokp.tile([128, N_CHUNKS, E], fp32, tag="g2")
            nc.vector.tensor_tensor(
                g2[:], mask2[:], p2[:].unsqueeze(2).broadcast_to((128, N_CHUNKS, E)),
                op=Alu.mult,
            )
            gates_sb = outp.tile([128, N_CHUNKS, E], fp32, tag="gates")
            nc.vector.tensor_tensor(gates_sb[:], g1[:], g2[:], op=Alu.add)
            nc.sync.dma_start(
                gates_d[tok_sl, :].rearrange("(j p) e -> p j e", p=128), gates_sb[:]
            )

            # pack + cast indices to int32 [128, 4, 2]
            idxf = smallp.tile([128, N_CHUNKS, 2], fp32, tag="idxf")
            nc.vector.tensor_copy(idxf[:, :, 0], i1[:])
            nc.vector.tensor_copy(idxf[:, :, 1], i2[:])
            idxi = outp.tile([128, N_CHUNKS, 2], i32, tag="idxi")
            nc.vector.tensor_copy(idxi[:], idxf[:])
            nc.sync.dma_start(
                idx_d[tok_sl, :].rearrange("(j p) k -> p j k", p=128), idxi[:]
            )

    nc.compile()
    return nc


def _host_prep(Wp, bp, W1, b1, W2, b2, Wc, bc, temperature):
    """Fold the linear chain + temperature into two weight matrices."""
    f64 = np.float64
    T = float(np.clip(np.asarray(temperature, f64).reshape(-1)[0], 0.5, 5.0))
    inv = 1.0 / T

    W1bd = np.zeros((F, F), f64)
    W2bd = np.zeros((F, F), f64)
    for h in range(4):
        s = slice(64 * h, 64 * (h + 1))
        W1bd[s, s] = np.asarray(W1[h], f64)
        W2bd[s, s] = np.asarray(W2[h], f64)

    Wpf = (np.asarray(Wp, f64) @ W1bd).astype(np.float32)          # [2048, 256]
    b1f = (np.asarray(bp, f64) @ W1bd + np.asarray(b1, f64).reshape(-1)).astype(
        np.float32
    )                                                               # [256]
    Wf = (W2bd @ np.asarray(Wc, f64) * inv).astype(np.float32)      # [256, 64]
    bcf = (
        (np.asarray(b2, f64).reshape(-1) @ np.asarray(Wc, f64) + np.asarray(bc, f64))
        * inv
    ).astype(np.float32)                                            # [64]

    wpf_arr = np.ascontiguousarray(
        Wpf.reshape(KT, 128, 2, 128).transpose(1, 0, 2, 3)
    )                                                               # [128,16,2,128]
    b1f_arr = np.ascontiguousarray(b1f.reshape(2, 128).T)           # [128, 2]
    wf_arr = np.ascontiguousarray(Wf.reshape(2, 128, E).transpose(1, 0, 2))
    bcf_arr = np.ascontiguousarray(bcf.reshape(E, 1))
    iota_arr = np.ascontiguousarray(
        np.broadcast_to(
            (np.arange(E, dtype=np.float32) + np.float32(BIG))[None, None, :],
            (128, N_CHUNKS, E),
        )
    )
    ident = np.eye(E, dtype=np.float32)
    return wpf_arr, b1f_arr, wf_arr, bcf_arr, iota_arr, ident


def _run(x, Wp, bp, W1, b1, W2, b2, Wc, bc, temperature, **run_kwargs):
    from concourse.bass_utils import run_bass_kernel_spmd

    x = np.asarray(x, np.float32)
    assert x.shape == (N_TOKENS, INPUT_DIM)

    wpf_arr, b1f_arr, wf_arr, bcf_arr, iota_arr, ident = _host_prep(
        Wp, bp, W1, b1, W2, b2, Wc, bc, temperature
    )

    if "nc" not in _CACHE:
        _CACHE["nc"] = _build_bass()
    nc = _CACHE["nc"]

    xt_full = x.T  # [2048, 16384] view
    in_maps = []
    for c in range(N_CORES):
        xt_c = np.ascontiguousarray(
            xt_full[:, c * N_SHARD : (c + 1) * N_SHARD]
        )
        in_maps.append(
            {
                "xt": xt_c,
                "wpf": wpf_arr,
                "b1f": b1f_arr,
                "wf": wf_arr,
                "bcf": bcf_arr,
                "iota": iota_arr,
                "ident": ident,
            }
        )

    res = run_bass_kernel_spmd(
        nc, in_maps, core_ids=list(range(N_CORES)), **run_kwargs
    )
    results = res.results

    gates = np.concatenate([r["gates"] for r in results], axis=0)
    top_idx = np.concatenate([r["idx"] for r in results], axis=0)
    gate_logits = np.concatenate([r["gl"] for r in results], axis=0)
    return (gates, top_idx, gate_logits), res


def kernel(x, Wp, bp, W1, b1, W2, b2, Wc, bc, temperature, noise_scale=None):
    out, _ = _run(x, Wp, bp, W1, b1, W2, b2, Wc, bc, temperature)
    return out


# revision 30
# speedup vs baseline: 4.8999x; 4.8999x over previous
"""MoE DynamicGate kernel for 8x Trainium2 NeuronCores.

Data-parallel over the token dim (16384 tokens -> 8 shards of 2048); all gate
parameters are tiny and replicated to every core.

Math (eval-mode forward, matching the nn.Module reference):
    h   = x @ Wp + bp                      [N, 256]   (viewed as 4 heads x 64)
    hid = relu(h @ blockdiag(W1) + b1)     [N, 256]
    cmb = hid @ blockdiag(W2) + b2         [N, 256]
    gl  = (cmb @ Wc + bc) / clip(T,.5,5)   [N, 64]
    top2 + softmax-over-2 + scatter

Since relu is the only nonlinearity, the host folds the linear pieces:
    Wpf = Wp @ blockdiag(W1)       bpf = bp @ blockdiag(W1) + b1
    Wf  = blockdiag(W2) @ Wc / T   bcf = (b2 @ Wc + bc) / T
so the device computes:  gl = relu(x @ Wpf + bpf) @ Wf + bcf
(T = 2.0 is a power of two so folding 1/T into Wf/bcf is bit-exact; for other
T it only perturbs at the 2^-24 level.)

The device works feature-major (x^T is prepared host-side) so every matmul has
its contraction dim on SBUF partitions, biases are per-partition (fused into
the ACT relu), and only the tiny [64, 512] logit tiles need a PE transpose
back to token-major for the top-k stage.
"""

import numpy as np

N_TOKENS = 16384
INPUT_DIM = 2048
N_CORES = 8
N_SHARD = N_TOKENS // N_CORES  # 2048
CHUNK = 512                    # tokens per pipeline chunk
N_CHUNKS = N_SHARD // CHUNK    # 4
KT = INPUT_DIM // 128          # 16 contraction tiles
F = 256                        # fused hidden width (4 heads x 64)
E = 64                         # experts
BIG = 1000.0

_CACHE = {}


def _build_bass(repeats=1, use_f32r=False, _skip_topk=False, _skip_mm=False,
                _xt_bufs=2):
    """Build the per-core Bass/Tile program (SPMD: identical on all cores).

    repeats>1 re-runs the whole computation that many times in one NEFF —
    only used for slope-based HW timing (dispatch overhead cancels).
    use_f32r runs the big x@Wpf matmul with float32r operands (full-rate PE
    instead of 1/4-rate fp32, at reduced multiply precision).
    """
    from contextlib import ExitStack

    import concourse.bass as bass
    import concourse.tile as tile
    from concourse import bacc, mybir

    fp32 = mybir.dt.float32
    fmm = mybir.dt.float32r if use_f32r else fp32
    i32 = mybir.dt.int32
    AF = mybir.ActivationFunctionType
    Alu = mybir.AluOpType
    AX = mybir.AxisListType

    nc = bacc.Bacc(
        "TRN2", target_bir_lowering=False, debug=False, num_devices=N_CORES
    )

    # DRAM I/O (host pre-arranges weight layouts for single clean DMAs)
    xt_d = nc.dram_tensor("xt", [INPUT_DIM, N_SHARD], fmm, kind="ExternalInput")
    wpf_d = nc.dram_tensor("wpf", [128, KT, 2, 128], fmm, kind="ExternalInput")
    b1f_d = nc.dram_tensor("b1f", [128, 2], fp32, kind="ExternalInput")
    wf_d = nc.dram_tensor("wf", [128, 2, E], fp32, kind="ExternalInput")
    bcf_d = nc.dram_tensor("bcf", [128, E], fp32, kind="ExternalInput")
    iota_d = nc.dram_tensor("iota", [128, N_CHUNKS, E], fp32, kind="ExternalInput")

    # single packed output: cols [0:64] gates, [64:128] gate_logits,
    # [128:130] top-2 indices (int32 bits stored raw in the f32 payload)
    OUTW = 2 * E + 2
    out_d = nc.dram_tensor("out", [N_SHARD, OUTW], fp32, kind="ExternalOutput")

    with tile.TileContext(nc) as tc, ExitStack() as ctx:
        consts = ctx.enter_context(tc.tile_pool(name="consts", bufs=1))
        xtp = ctx.enter_context(tc.tile_pool(name="xt", bufs=_xt_bufs))
        hidp = ctx.enter_context(tc.tile_pool(name="hid", bufs=2))
        tokp = ctx.enter_context(tc.tile_pool(name="tok", bufs=2))
        outp = ctx.enter_context(tc.tile_pool(name="outs", bufs=2))
        smallp = ctx.enter_context(tc.tile_pool(name="small", bufs=2))
        psA = ctx.enter_context(
            tc.tile_pool(name="psA", bufs=4, space=bass.MemorySpace.PSUM)
        )
        psT = ctx.enter_context(
            tc.tile_pool(name="psT", bufs=2, space=bass.MemorySpace.PSUM)
        )

        # constant loads
        wpf_sb = consts.tile([128, KT, 2, 128], fmm)
        nc.sync.dma_start(wpf_sb[:], wpf_d[:])
        b1f_sb = consts.tile([128, 2], fp32)
        nc.sync.dma_start(b1f_sb[:], b1f_d[:])
        wf_sb = consts.tile([128, 2, E], fp32)
        nc.sync.dma_start(wf_sb[:], wf_d[:])
        bcf_sb = consts.tile([128, E], fp32)
        nc.sync.dma_start(bcf_sb[:], bcf_d[:])
        iota_sb = consts.tile([128, N_CHUNKS, E], fp32)
        nc.sync.dma_start(iota_sb[:], iota_d[:])

        for c in [ci for _ in range(repeats) for ci in range(N_CHUNKS)]:
            tok_sl = slice(c * CHUNK, (c + 1) * CHUNK)

            # x^T chunk: [128 part (d), 16 ktiles, 512 tokens]
            xt = xtp.tile([128, KT, CHUNK], fmm, tag="xt")
            nc.sync.dma_start(
                xt[:], xt_d[:, tok_sl].rearrange("(k p) t -> p k t", p=128)
            )

            # hid_pre^T = (x @ Wpf)^T by 128-feat halves, K-contiguous
            hids = []
            for m in range(2):
                ps = psA.tile([128, CHUNK], fp32, tag="mm")
                for k in range(1 if _skip_mm else KT):
                    nc.tensor.matmul(
                        ps[:],
                        wpf_sb[:, k, m, :],
                        xt[:, k, :],
                        start=(k == 0),
                        stop=(k == (0 if _skip_mm else KT - 1)),
                    )
                hid = hidp.tile([128, CHUNK], fp32, tag=f"hid{m}")
                nc.scalar.activation(
                    hid[:], ps[:], AF.Relu, bias=b1f_sb[:, m : m + 1]
                )
                hids.append(hid)

            # gate logits directly token-major: hid slices are the stationary
            # operand (lhsT = hid[:, 128j:128j+128] is [feat, token]), Wf
            # halves stream -> psum [128 tokens, 64 experts] per j-block.
            pt = psT.tile([128, N_CHUNKS, E], fp32, tag="pt")
            for j in range(N_CHUNKS):
                nc.tensor.matmul(
                    pt[:, j, :], hids[0][:, j * 128 : (j + 1) * 128],
                    wf_sb[:, 0, :], start=True, stop=False,
                )
                nc.tensor.matmul(
                    pt[:, j, :], hids[1][:, j * 128 : (j + 1) * 128],
                    wf_sb[:, 1, :], start=False, stop=True,
                )
            # packed per-chunk output staging [128 tok, 4 j, 130]
            packed = outp.tile([128, N_CHUNKS, OUTW], fp32, tag="packed")
            gl_tok = packed[:, :, E : 2 * E]
            # add bias (replicated across partitions) while copying PSUM->SBUF
            nc.vector.tensor_tensor(
                gl_tok, pt[:],
                bcf_sb[:].unsqueeze(1).broadcast_to((128, N_CHUNKS, E)),
                op=Alu.add,
            )

            if _skip_topk:
                nc.sync.dma_start(
                    out_d[tok_sl, :].rearrange("(j p) e -> p j e", p=128),
                    packed[:],
                )
                continue
            # ---- top-2 + softmax + scatter-free gates ----
            L = gl_tok
            m1 = smallp.tile([128, N_CHUNKS], fp32, tag="m1")
            nc.vector.reduce_max(m1[:], L, axis=AX.X)
            m1b = m1[:].unsqueeze(2).broadcast_to((128, N_CHUNKS, E))
            mask1 = tokp.tile([128, N_CHUNKS, E], fp32, tag="mask1")
            nc.vector.tensor_tensor(mask1[:], L, m1b, op=Alu.is_ge)

            L2 = tokp.tile([128, N_CHUNKS, E], fp32, tag="L2")
            nc.vector.scalar_tensor_tensor(
                L2[:], mask1[:], -BIG, L, op0=Alu.mult, op1=Alu.add
            )
            m2 = smallp.tile([128, N_CHUNKS], fp32, tag="m2")
            nc.vector.reduce_max(m2[:], L2[:], axis=AX.X)
            m2b = m2[:].unsqueeze(2).broadcast_to((128, N_CHUNKS, E))
            mask2 = tokp.tile([128, N_CHUNKS, E], fp32, tag="mask2")
            nc.vector.tensor_tensor(mask2[:], L2[:], m2b, op=Alu.is_ge)

            # argmax indices: min over (iota + BIG - BIG*mask)
            t1 = tokp.tile([128, N_CHUNKS, E], fp32, tag="t1")
            nc.vector.scalar_tensor_tensor(
                t1[:], mask1[:], -BIG, iota_sb[:], op0=Alu.mult, op1=Alu.add
            )
            i1 = smallp.tile([128, N_CHUNKS], fp32, tag="i1")
            nc.vector.tensor_reduce(i1[:], t1[:], axis=AX.X, op=Alu.min)
            t2 = tokp.tile([128, N_CHUNKS, E], fp32, tag="t2")
            nc.vector.scalar_tensor_tensor(
                t2[:], mask2[:], -BIG, iota_sb[:], op0=Alu.mult, op1=Alu.add
            )
            i2 = smallp.tile([128, N_CHUNKS], fp32, tag="i2")
            nc.vector.tensor_reduce(i2[:], t2[:], axis=AX.X, op=Alu.min)

            # softmax over the two top values: p1 = 1/(1+exp(m2-m1)), p2 = e*p1
            d = smallp.tile([128, N_CHUNKS], fp32, tag="d")
            nc.vector.tensor_tensor(d[:], m2[:], m1[:], op=Alu.subtract)
            ex = smallp.tile([128, N_CHUNKS], fp32, tag="ex")
            nc.scalar.activation(ex[:], d[:], AF.Exp)
            den = smallp.tile([128, N_CHUNKS], fp32, tag="den")
            nc.vector.tensor_scalar_add(den[:], ex[:], 1.0)
            p1 = smallp.tile([128, N_CHUNKS], fp32, tag="p1")
            nc.vector.reciprocal(p1[:], den[:])
            p2 = smallp.tile([128, N_CHUNKS], fp32, tag="p2")
            nc.vector.tensor_tensor(p2[:], ex[:], p1[:], op=Alu.mult)

            g1 = tokp.tile([128, N_CHUNKS, E], fp32, tag="g1")
            nc.vector.tensor_tensor(
                g1[:], mask1[:], p1[:].unsqueeze(2).broadcast_to((128, N_CHUNKS, E)),
                op=Alu.mult,
            )
            g2 = tokp.tile([128, N_CHUNKS, E], fp32, tag="g2")
            nc.vector.tensor_tensor(
                g2[:], mask2[:], p2[:].unsqueeze(2).broadcast_to((128, N_CHUNKS, E)),
                op=Alu.mult,
            )
            nc.vector.tensor_tensor(packed[:, :, 0:E], g1[:], g2[:], op=Alu.add)

            # indices: convert f32->int32 into the packed payload (raw bits)
            idx_view = packed[:, :, 2 * E : 2 * E + 2].bitcast(i32)
            nc.vector.tensor_copy(idx_view[:, :, 0], i1[:])
            nc.vector.tensor_copy(idx_view[:, :, 1], i2[:])

            nc.sync.dma_start(
                out_d[tok_sl, :].rearrange("(j p) e -> p j e", p=128), packed[:]
            )

    nc.compile()
    return nc


def _host_prep(Wp, bp, W1, b1, W2, b2, Wc, bc, temperature):
    """Fold the linear chain + temperature into two weight matrices."""
    f64 = np.float64
    T = float(np.clip(np.asarray(temperature, f64).reshape(-1)[0], 0.5, 5.0))
    inv = 1.0 / T

    W1bd = np.zeros((F, F), f64)
    W2bd = np.zeros((F, F), f64)
    for h in range(4):
        s = slice(64 * h, 64 * (h + 1))
        W1bd[s, s] = np.asarray(W1[h], f64)
        W2bd[s, s] = np.asarray(W2[h], f64)

    Wpf = (np.asarray(Wp, f64) @ W1bd).astype(np.float32)          # [2048, 256]
    b1f = (np.asarray(bp, f64) @ W1bd + np.asarray(b1, f64).reshape(-1)).astype(
        np.float32
    )                                                               # [256]
    Wf = (W2bd @ np.asarray(Wc, f64) * inv).astype(np.float32)      # [256, 64]
    bcf = (
        (np.asarray(b2, f64).reshape(-1) @ np.asarray(Wc, f64) + np.asarray(bc, f64))
        * inv
    ).astype(np.float32)                                            # [64]

    wpf_arr = np.ascontiguousarray(
        Wpf.reshape(KT, 128, 2, 128).transpose(1, 0, 2, 3)
    )                                                               # [128,16,2,128]
    b1f_arr = np.ascontiguousarray(b1f.reshape(2, 128).T)           # [128, 2]
    wf_arr = np.ascontiguousarray(Wf.reshape(2, 128, E).transpose(1, 0, 2))
    bcf_arr = np.ascontiguousarray(np.broadcast_to(bcf[None, :], (128, E)))
    iota_arr = np.ascontiguousarray(
        np.broadcast_to(
            (np.arange(E, dtype=np.float32) + np.float32(BIG))[None, None, :],
            (128, N_CHUNKS, E),
        )
    )
    return wpf_arr, b1f_arr, wf_arr, bcf_arr, iota_arr


def _run(x, Wp, bp, W1, b1, W2, b2, Wc, bc, temperature, **run_kwargs):
    from concourse.bass_utils import run_bass_kernel_spmd

    x = np.asarray(x, np.float32)
    assert x.shape == (N_TOKENS, INPUT_DIM)

    wpf_arr, b1f_arr, wf_arr, bcf_arr, iota_arr = _host_prep(
        Wp, bp, W1, b1, W2, b2, Wc, bc, temperature
    )

    if "nc" not in _CACHE:
        _CACHE["nc"] = _build_bass()
    nc = _CACHE["nc"]

    xt_full = x.T  # [2048, 16384] view
    in_maps = []
    for c in range(N_CORES):
        xt_c = np.ascontiguousarray(
            xt_full[:, c * N_SHARD : (c + 1) * N_SHARD]
        )
        in_maps.append(
            {
                "xt": xt_c,
                "wpf": wpf_arr,
                "b1f": b1f_arr,
                "wf": wf_arr,
                "bcf": bcf_arr,
                "iota": iota_arr,
            }
        )

    res = run_bass_kernel_spmd(
        nc, in_maps, core_ids=list(range(N_CORES)), **run_kwargs
    )
    packed = np.concatenate([r["out"] for r in res.results], axis=0)

    gates = np.ascontiguousarray(packed[:, :E])
    gate_logits = np.ascontiguousarray(packed[:, E : 2 * E])
    top_idx = np.ascontiguousarray(packed[:, 2 * E : 2 * E + 2]).view(np.int32)
    return (gates, top_idx, gate_logits), res


def kernel(x, Wp, bp, W1, b1, W2, b2, Wc, bc, temperature, noise_scale=None):
    out, _ = _run(x, Wp, bp, W1, b1, W2, b2, Wc, bc, temperature)
    return out
